# revision 11
# baseline (speedup 1.0000x reference)
"""AttentionPooling (segment softmax pooling) Trainium2 kernel.

Math (per reference):
    h = tanh(x @ W1 + b1); s = h @ W2 + b2
    w = softmax(s) within each contiguous segment (batch is sorted)
    out[b] = sum_{r in b} w_r * x[r]

Device algorithm (per core, segments sharded 512/core):
  Softmax is shift-invariant and |s| <= ||W2||_1 + |b2| ~ 9, so we skip the
  per-segment max and use e_r = exp(s_r + b2) directly (safe in fp32).
  out[b] = (sum e_r x_r) / (sum e_r): both sums come from one-hot matmuls
  contracted over rows, accumulated in PSUM over a w_seg-segment group
  window, then scatter-accumulated (indirect DMA, compute_op=add) into a
  DRAM scratch [segs, 257] (256 pooled cols + 1 sum col); block b of the
  final divide pass is interleaved into the main loop as soon as no later
  group window can touch it.

  Scores per group: u = x@W1 via xT tiles (bf16), tanh on the scalar
  engine, then s = W2^T h as four 1-row matmuls landing on PSUM partitions
  {0,32,64,96} of one bank; exp runs on that [128,512] tile in one
  activation, and a DRAM round-trip (2 tiny DMAs) re-partitions e into
  [128, 16] row-natural order.

  The one-hot window masks (onehot[r, s] = [segment(r) == window_start+s])
  depend only on `batch`, so the host precomputes them in fp8e4 (0/1 exact)
  and the device just scales them by e_r (scalar-engine copy / vector
  tensor_scalar, split per tile) to form the pooling matmul weights.

The program is identical across cores (SPMD); all data-dependent segment
offsets travel through input tensors (onehot cols + scatter row indices).
The normalize-stage schedule is the max over cores, so it is core-uniform.
"""

import os
from contextlib import ExitStack

import numpy as np
import ml_dtypes

LAST_EXEC_NS = None

import concourse.bass as bass
import concourse.bacc as bacc
import concourse.tile as tile
from concourse import mybir
from concourse.bass import IndirectOffsetOnAxis
from concourse.bass_utils import run_bass_kernel_spmd

# ---- problem constants (hardcoded per contract) ----
N_TOTAL = 500000
D = 256
H = 128
NUM_SEGMENTS = 4096
N_CORES = 8
SEGS_PER_CORE = NUM_SEGMENTS // N_CORES  # 512

G_ROWS = 2048          # rows per group
TILES_PER_G = 16       # 128-row tiles per group
SUB_PER_G = 4          # 512-row subtiles per group (score matmuls)
SCRATCH_ROWS = 640     # 512 real segs + pad rows for window overflow

F32 = mybir.dt.float32
BF16 = mybir.dt.bfloat16
FP8 = mybir.dt.float8e4
I32 = mybir.dt.int32


def build_nc(n_groups: int, b2_val: float, w_seg: int, norm_after: list[int]) -> bass.Bass:
    """norm_after[b] = group index after whose scatter output block b
    (scratch rows [128b, 128b+128)) is final on every core."""
    r_pad = n_groups * G_ROWS
    n_tiles = n_groups * TILES_PER_G

    nc = bacc.Bacc("TRN2", target_bir_lowering=False, debug=False)

    # DRAM I/O
    # x_nat carries D cols of x, a ones column (col 256, folds the seg_sum
    # matmul into the pooling matmul), and a zero pad col. Layout is
    # partition-major [128, n_tiles, 258]: x_nat[p, t, :] = row 128t + p.
    # oh_nat carries the w_seg one-hot window cols in fp8 (0/1 exact).
    x_nat = nc.dram_tensor("x_nat", [128, n_tiles, D + 2], BF16, kind="ExternalInput")
    oh_nat = nc.dram_tensor("oh_nat", [128, n_tiles, w_seg], FP8, kind="ExternalInput")
    xT = nc.dram_tensor("xT", [D, r_pad], BF16, kind="ExternalInput")
    w1c = nc.dram_tensor("w1c", [2, 128, H], BF16, kind="ExternalInput")
    w2col = nc.dram_tensor("w2col", [H, 1], BF16, kind="ExternalInput")
    b1col = nc.dram_tensor("b1col", [H, 1], F32, kind="ExternalInput")
    seg_idx = nc.dram_tensor("seg_idx", [w_seg, n_groups], I32, kind="ExternalInput")
    # per-group staging row for the e re-partition round-trip
    # (ExternalOutput so the PJRT path gives it a real per-core allocation)
    e_stage = nc.dram_tensor("e_stage", [n_groups, G_ROWS], F32, kind="ExternalOutput")
    # ExternalOutput buffers are zero-initialized by the runtime — scratch
    # relies on that for its scatter-accumulate
    scratch = nc.dram_tensor("scratch", [SCRATCH_ROWS, 257], F32, kind="ExternalOutput")
    out = nc.dram_tensor("out", [SEGS_PER_CORE, D], F32, kind="ExternalOutput")

    with tile.TileContext(nc) as tc, ExitStack() as ctx:
        const_pool = ctx.enter_context(tc.tile_pool(name="const", bufs=1))
        xT_pool = ctx.enter_context(tc.tile_pool(name="xT", bufs=10))
        xnat_pool = ctx.enter_context(tc.tile_pool(name="xnat", bufs=8))
        oh_pool = ctx.enter_context(tc.tile_pool(name="oh", bufs=8))
        h_pool = ctx.enter_context(tc.tile_pool(name="h", bufs=6))
        esb_pool = ctx.enter_context(tc.tile_pool(name="esb", bufs=4))
        e_pool = ctx.enter_context(tc.tile_pool(name="e", bufs=4))
        esel_pool = ctx.enter_context(tc.tile_pool(name="esel", bufs=12))
        flush_pool = ctx.enter_context(tc.tile_pool(name="flush", bufs=2))
        fin_pool = ctx.enter_context(tc.tile_pool(name="fin", bufs=4))
        u_psum = ctx.enter_context(tc.tile_pool(name="u_ps", bufs=2, space="PSUM"))
        s_psum = ctx.enter_context(tc.tile_pool(name="s_ps", bufs=2, space="PSUM"))
        p_psum = ctx.enter_context(tc.tile_pool(name="p_ps", bufs=2, space="PSUM"))

        # ---- constants ----
        w1c_t = const_pool.tile([128, 2 * H], BF16, tag="w1c")
        nc.sync.dma_start(w1c_t[:, 0:H], w1c[0])
        nc.sync.dma_start(w1c_t[:, H : 2 * H], w1c[1])
        w2_t = const_pool.tile([H, 1], BF16, tag="w2")
        nc.sync.dma_start(w2_t[:], w2col[:, :])
        b1_t = const_pool.tile([H, 1], F32, tag="b1")
        nc.sync.dma_start(b1_t[:], b1col[:, :])
        sidx_t = const_pool.tile([w_seg, n_groups], I32, tag="sidx")
        nc.sync.dma_start(sidx_t[:], seg_idx[:, :])

        def normalize_block(b: int):
            # out[s] = scratch[s, :256] / scratch[s, 256] for rows [128b, 128b+128)
            ft = fin_pool.tile([128, 257], F32, tag="ft")
            nc.gpsimd.dma_start(ft[:], scratch[128 * b : 128 * (b + 1), :])
            rec = fin_pool.tile([128, 1], F32, tag="rec")
            eps = fin_pool.tile([128, 1], F32, tag="eps")
            nc.vector.tensor_scalar(
                eps[:], ft[:, D : D + 1], 1e-30, None, mybir.AluOpType.add,
            )
            nc.vector.reciprocal(rec[:], eps[:])
            ot = fin_pool.tile([128, D], F32, tag="ot")
            nc.vector.tensor_scalar(
                ot[:], ft[:, 0:D], rec[:, 0:1], None, mybir.AluOpType.mult,
            )
            nc.sync.dma_start(out[128 * b : 128 * (b + 1), :], ot[:])

        # ---- main loop over row groups ----
        for g in range(n_groups):
            xt0 = xT_pool.tile([128, G_ROWS], BF16, tag="xt0")
            xt1 = xT_pool.tile([128, G_ROWS], BF16, tag="xt1")
            nc.sync.dma_start(xt0[:], xT[0:128, g * G_ROWS : (g + 1) * G_ROWS])
            nc.sync.dma_start(xt1[:], xT[128:256, g * G_ROWS : (g + 1) * G_ROWS])

            # scores: e_nat[p, c] = exp(score(row 2048g + 128c + p) + b2); two
            # PSUM tiles (banks) so each half's exp can fire without waiting
            e_nat = e_pool.tile([128, TILES_PER_G], F32, tag="e_nat")
            for half in range(2):
                snat = s_psum.tile([128, 8], F32, tag=f"snat{half}")
                for ii in range(SUB_PER_G // 2):
                    i = 2 * half + ii
                    sl = slice(512 * i, 512 * (i + 1))
                    u = u_psum.tile([H, 512], F32, tag="u")
                    nc.tensor.matmul(u[:], w1c_t[:, 0:H], xt0[:, sl], start=True, stop=False)
                    nc.tensor.matmul(u[:], w1c_t[:, H : 2 * H], xt1[:, sl], start=False, stop=True)
                    h_t = h_pool.tile([H, 512], BF16, tag="h")
                    nc.scalar.activation(h_t[:], u[:], mybir.ActivationFunctionType.Tanh, bias=b1_t[:, 0:1])
                    for j in range(4):
                        lc = 4 * ii + j
                        nc.tensor.matmul(
                            snat[:, lc : lc + 1],
                            h_t[:, 128 * j : 128 * (j + 1)],
                            w2_t[:],
                            start=(lc == 0),
                            stop=(lc == 7),
                            skip_group_check=True,
                        )
                nc.scalar.activation(
                    e_nat[:, 8 * half : 8 * (half + 1)],
                    snat[:],
                    mybir.ActivationFunctionType.Exp,
                    bias=float(b2_val),
                )

            # pooling: accumulate [w_seg segs, 256 pooled + 1 sum] over the group
            pooled = p_psum.tile([128, 257], F32, tag="pooled")
            xn = xnat_pool.tile([128, TILES_PER_G * (D + 2)], BF16, tag="xn")
            t0 = g * TILES_PER_G
            nc.scalar.dma_start(
                xn[:].rearrange("p (t d) -> p t d", d=D + 2),
                x_nat[:, t0 : t0 + TILES_PER_G, :],
            )
            # (xn stays on the scalar HWDGE queue: sync carries xT + e_stage)
            oh = oh_pool.tile([128, TILES_PER_G * w_seg], FP8, tag="oh")
            nc.gpsimd.dma_start(
                oh[:].rearrange("p (t w) -> p t w", w=w_seg),
                oh_nat[:, t0 : t0 + TILES_PER_G, :],
            )
            for c in range(TILES_PER_G):
                ohs = oh[:, c * w_seg : (c + 1) * w_seg]
                esel = esel_pool.tile([128, w_seg], BF16, tag="esel")
                # esel = e[row] * onehot[row, seg]; split scalar/vector engines
                if c % 8 == 0:
                    nc.scalar.mul(esel[:], ohs, e_nat[:, c : c + 1])
                else:
                    nc.vector.tensor_scalar_mul(esel[:], ohs, e_nat[:, c : c + 1])
                rhs = xn[:, c * (D + 2) : c * (D + 2) + 257]
                nc.tensor.matmul(
                    pooled[0:w_seg, 0:257], esel[:], rhs,
                    start=(c == 0), stop=(c == TILES_PER_G - 1),
                    skip_group_check=True,
                )
            # flush: psum -> sbuf -> scatter-accumulate into scratch rows
            fl = flush_pool.tile([w_seg, 257], F32, tag="fl")
            nc.vector.tensor_copy(fl[:], pooled[0:w_seg, :])
            nc.gpsimd.indirect_dma_start(
                scratch[:, :],
                IndirectOffsetOnAxis(ap=sidx_t[:, g : g + 1], axis=0),
                fl[:],
                None,
                compute_op=mybir.AluOpType.add,
            )
            for b in range(4):
                if norm_after[b] == g:
                    normalize_block(b)

    return nc


def kernel(x, batch, W1, b1, W2, b2):
    x = np.asarray(x, dtype=np.float32)
    batch = np.asarray(batch)
    W1 = np.asarray(W1, dtype=np.float32)
    b1 = np.asarray(b1, dtype=np.float32)
    W2 = np.asarray(W2, dtype=np.float32)
    b2 = np.asarray(b2, dtype=np.float32)
    n, d = x.shape
    assert d == D

    bounds = np.searchsorted(batch, np.arange(NUM_SEGMENTS + 1))
    core_starts = [int(bounds[SEGS_PER_CORE * m]) for m in range(N_CORES + 1)]
    rows_per_core = [core_starts[m + 1] - core_starts[m] for m in range(N_CORES)]
    n_groups = max(1, int(np.ceil(max(rows_per_core) / G_ROWS)))
    r_pad = n_groups * G_ROWS
    n_tiles = n_groups * TILES_PER_G

    # window width: per-group segment span (exact start); round up to x4
    max_span = 0
    for m in range(N_CORES):
        rs, re = core_starts[m], core_starts[m + 1]
        seg_local = (batch[rs:re] - SEGS_PER_CORE * m).astype(np.int64)
        for g in range(n_groups):
            lo, hi = g * G_ROWS, min((g + 1) * G_ROWS, re - rs)
            if lo >= hi:
                continue
            span = int(seg_local[hi - 1]) - int(seg_local[lo]) + 1
            max_span = max(max_span, span)
    w_seg = max(16, 4 * int(np.ceil((max_span + 1) / 4)))
    assert w_seg <= 128, f"group seg span {max_span} too large"

    # normalize-stage schedule: block b (segs [128b,128b+128)) is final after
    # group g iff every later group's window starts at or beyond 128(b+1).
    # Conservative across cores so the SPMD program is core-uniform.
    norm_after = [n_groups - 1] * 4
    all_s0 = np.full((N_CORES, n_groups), SEGS_PER_CORE, dtype=np.int64)

    # shared constant inputs
    w1c = np.ascontiguousarray(W1.reshape(2, 128, H).astype(ml_dtypes.bfloat16))
    w2col = np.ascontiguousarray(W2.reshape(H, 1).astype(ml_dtypes.bfloat16))
    b1col = np.ascontiguousarray(b1.reshape(H, 1))
    b2_val = float(b2.reshape(-1)[0])

    in_maps = []
    for m in range(N_CORES):
        rs, re = core_starts[m], core_starts[m + 1]
        rows = re - rs
        xm = x[rs:re]
        x_flat = np.zeros((r_pad, D + 2), dtype=ml_dtypes.bfloat16)
        x_flat[:rows, :D] = xm.astype(ml_dtypes.bfloat16)
        x_flat[:rows, D] = ml_dtypes.bfloat16(1.0)
        oh_flat = np.zeros((r_pad, w_seg), dtype=ml_dtypes.float8_e4m3fn)

        seg_local = (batch[rs:re] - SEGS_PER_CORE * m).astype(np.int64)
        assert seg_local.min() >= 0 and seg_local.max() < SEGS_PER_CORE

        sidx = np.empty((w_seg, n_groups), dtype=np.int32)
        for g in range(n_groups):
            lo = g * G_ROWS
            hi = min((g + 1) * G_ROWS, rows)
            if lo >= rows:
                s0 = SEGS_PER_CORE  # pad region
            else:
                s0 = int(seg_local[lo])
                rr = np.arange(lo, hi)
                oh_flat[rr, seg_local[lo:hi] - s0] = ml_dtypes.float8_e4m3fn(1.0)
            sidx[:, g] = s0 + np.arange(w_seg)
            all_s0[m, g] = s0

        # partition-major: x_nat[p, t, :] = x_flat[128t + p, :]
        x_nat = np.ascontiguousarray(
            x_flat.reshape(n_tiles, 128, D + 2).transpose(1, 0, 2)
        )
        oh_nat = np.ascontiguousarray(
            oh_flat.reshape(n_tiles, 128, w_seg).transpose(1, 0, 2)
        )
        xT = np.zeros((D, r_pad), dtype=ml_dtypes.bfloat16)
        xT[:, :rows] = xm.T.astype(ml_dtypes.bfloat16)

        in_maps.append(
            {
                "x_nat": x_nat,
                "oh_nat": oh_nat,
                "xT": xT,
                "w1c": w1c,
                "w2col": w2col,
                "b1col": b1col,
                "seg_idx": sidx,
            }
        )

    for b in range(4):
        # first g such that every group AFTER g has s0 >= 128(b+1) + nothing
        # pending; windows may extend w_seg below their s0 start row.
        need = 128 * (b + 1)
        gb = 0
        for m in range(N_CORES):
            later_touch = n_groups - 1
            for g in range(n_groups - 1, -1, -1):
                if all_s0[m, g] >= need:
                    later_touch = g - 1
                else:
                    break
            gb = max(gb, later_touch + 1 if later_touch >= 0 else 0)
        norm_after[b] = min(max(gb, 0), n_groups - 1)
    # monotone, and the last block always normalizes at the end
    for b in range(1, 4):
        norm_after[b] = max(norm_after[b], norm_after[b - 1])
    norm_after[3] = n_groups - 1

    nc = build_nc(n_groups, b2_val, w_seg, norm_after)
    if not nc.is_finalized():
        nc.finalize()
    trace = os.environ.get("KERNEL_TRACE", "0") == "1"
    kw = {}
    if trace:
        kw = dict(trace=True, tmpdir=os.environ.get("KERNEL_TRACE_DIR") or None)
    res = run_bass_kernel_spmd(nc, in_maps, core_ids=list(range(N_CORES)), **kw)
    global LAST_EXEC_NS
    LAST_EXEC_NS = res.exec_time_ns
    if trace:
        print(
            f"exec_time_ns={res.exec_time_ns} mean={res.mean_exec_time_ns} "
            f"max_core={res.max_exec_time_core_id}",
            flush=True,
        )
    outs = res.results

    full = np.empty((NUM_SEGMENTS, D), dtype=np.float32)
    for m in range(N_CORES):
        full[SEGS_PER_CORE * m : SEGS_PER_CORE * (m + 1)] = outs[m]["out"][
            :SEGS_PER_CORE
        ]
    return full
